# revision 2
# baseline (speedup 1.0000x reference)
"""Trainium2 Bass kernel for nn_BlockSparseMLP (MoE gated MLP, E=8, top-2).

Strategy: expert parallelism over 8 NeuronCores. The router matmul
(x @ w_router, 67 MFLOP out of the 206 GFLOP total) plus the top-2
dispatch/gather and the final scatter-add combine run on the host; each
core runs the full gated MLP (gate/up, silu*up, down, weighted by the
routing prob) for the tokens routed to its expert.

All matmul operands are bf16 (host-converted; PSUM accumulation stays
fp32), which runs the PE at full rate (1 row/cycle) like fp32r but
halves HBM traffic — per-core DMA drops from ~104 MB to ~54 MB, far
below the ~328 us of PE work, so the kernel is purely compute-bound.

Per-core device layout (capacity C = 512 tokens, token-major tiles):
  phase 1 (gate/up): stream w_gate/w_up in [128, 256] chunks packed as
    one [128, 512] tile; weights are stationary, xT moving (N=512).
    silu(gate)*up fused on ACT+DVE into aT ([I, C] bf16, resident).
  phase 2 (down): stream w_down in [128, 512] tiles (moving), aT tiles
    stationary, accumulate over I into [tokens, 512] psum tiles, scale
    by the per-token routing weight, DMA out fp32.
"""

import sys
import functools

sys.path.insert(0, "/opt/trn_rl_repo")

import numpy as np
import ml_dtypes

BF16 = ml_dtypes.bfloat16

T, H, II, E, TOPK = 2048, 2048, 4096, 8, 2
NCORES = 8
B0 = 512        # per-expert token capacity (moving N for gate/up)
CHUNK = 256     # phase-1 weight chunk width along I
KT = H // 128   # 16 contraction tiles for gate/up
MTI = II // 128  # 32 I tiles
NMC = II // CHUNK  # 16 weight chunks
JJ = CHUNK // 128  # 2 m-tiles per chunk
KI = II // 128  # 32 contraction tiles for down
NH = H // 512   # 4 output column chunks


@functools.lru_cache(maxsize=2)
def _build(nb1: int = 0):
    """Build the SPMD Bass program (capacity B0 tokens; nb1 kept for
    test.py signature compat and must be 0 — overflow spills to host)."""
    assert nb1 == 0
    import concourse.mybir as mybir
    import concourse.tile as tile
    from concourse import bacc

    f32 = mybir.dt.float32
    bf16 = mybir.dt.bfloat16
    NT = B0 // 128

    nc = bacc.Bacc(None)
    xT0 = nc.declare_dram_parameter("xT0", [KT, 128, B0], bf16, isOutput=False)
    wgu = nc.declare_dram_parameter("wgu", [NMC, KT, 128, 2 * CHUNK], bf16, isOutput=False)
    wd = nc.declare_dram_parameter("wd", [NH, KI, 128, 512], bf16, isOutput=False)
    rw = nc.declare_dram_parameter("rw", [128, NT], f32, isOutput=False)
    dout = nc.declare_dram_parameter("d", [NT, 128, H], f32, isOutput=True)

    SILU = mybir.ActivationFunctionType.Silu

    with tile.TileContext(nc) as tc:
        with (
            tc.tile_pool(name="pers", bufs=1) as pers,
            tc.tile_pool(name="wpool", bufs=32) as wpool,
        ):
            aT0 = pers.tile([128, MTI, B0], bf16)
            rwt = pers.tile([128, NT], f32)
            nc.gpsimd.dma_start(rwt[:], rw[:])

            with (
                tc.tile_pool(name="xp", bufs=1) as xp,
                tc.tile_pool(name="ps1", bufs=1, space="PSUM") as ps1,
                tc.tile_pool(name="sp", bufs=2) as sp,
            ):
                xt0 = [xp.tile([128, B0], bf16, name=f"xt0_{k}") for k in range(KT)]

                for mc in range(NMC):
                    # One [128, 512] tile holds this chunk of BOTH w_gate
                    # (cols 0:256) and w_up (cols 256:512). The xT loads are
                    # interleaved with mc0's weight loads on the opposite
                    # HWDGE queue so the k-loop's inputs arrive in
                    # consumption order instead of serializing the head.
                    wb = []
                    for k in range(KT):
                        eng_x = nc.sync if k % 2 == 0 else nc.scalar
                        eng_w = nc.scalar if k % 2 == 0 else nc.sync
                        if mc == 0:
                            eng_x.dma_start(xt0[k][:], xT0[k])
                        wbk = wpool.tile([128, 2 * CHUNK], bf16, name="wbk", tag="w")
                        eng_w.dma_start(wbk[:], wgu[mc, k])
                        wb.append(wbk)

                    pg0 = [ps1.tile([128, B0], f32, name="pg0", tag="pg0", bufs=3)
                           for _ in range(JJ)]
                    pu0 = [ps1.tile([128, B0], f32, name="pu0", tag="pu0", bufs=3)
                           for _ in range(JJ)]

                    for k in range(KT):
                        st = dict(start=(k == 0), stop=(k == KT - 1))
                        for j in range(JJ):
                            nc.tensor.matmul(
                                pg0[j][:], wb[k][:, j * 128:(j + 1) * 128], xt0[k][:], **st
                            )
                        for j in range(JJ):
                            nc.tensor.matmul(
                                pu0[j][:],
                                wb[k][:, CHUNK + j * 128:CHUNK + (j + 1) * 128],
                                xt0[k][:], **st
                            )

                    for j in range(JJ):
                        m = mc * JJ + j
                        sg = sp.tile([128, B0], f32, name="sg", tag="sg")
                        nc.scalar.activation(sg[:], pg0[j][:], SILU)
                        nc.vector.tensor_mul(aT0[:, m, :], sg[:], pu0[j][:])

            with (
                tc.tile_pool(name="ps2", bufs=1, space="PSUM") as ps2,
                tc.tile_pool(name="op", bufs=3) as op,
            ):
                pd_bufs = min(8, NT + 3)
                for nh in range(NH):
                    pd = [ps2.tile([128, 512], f32, name="pd", tag="pd", bufs=pd_bufs)
                          for _ in range(NT)]
                    for k in range(KI):
                        wdk = wpool.tile([128, 512], bf16, name="wdk", tag="w")
                        eng = nc.sync if k % 2 == 0 else nc.scalar
                        eng.dma_start(wdk[:], wd[nh, k])
                        st = dict(start=(k == 0), stop=(k == KI - 1))
                        for mt in range(NT):
                            nc.tensor.matmul(
                                pd[mt][:], aT0[:, k, mt * 128:(mt + 1) * 128],
                                wdk[:], **st
                            )
                    for mt in range(NT):
                        ot = op.tile([128, 512], f32, name="ot", tag="ot")
                        nc.vector.tensor_scalar_mul(ot[:], pd[mt][:], rwt[:, mt:mt + 1])
                        nc.gpsimd.dma_start(dout[mt][:, nh * 512:(nh + 1) * 512], ot[:])

    nc.compile()
    return nc


@functools.lru_cache(maxsize=2)
def _get_exec(nb1: int = 0):
    """Compile the Bass program and return (nc, run_fn) with a cached jit.

    run_fn(in_maps) -> list of per-core {"d": np.ndarray}. Mirrors
    bass2jax.run_bass_via_pjrt's multi-core branch, but keeps the jitted
    function alive across kernel() calls so repeat invocations skip XLA
    and NEFF compilation.
    """
    import jax
    import concourse.mybir as mybir
    from concourse import bass2jax

    nc = _build(nb1)
    bass2jax.install_neuronx_cc_hook()

    partition_name = nc.partition_id_tensor.name if nc.partition_id_tensor else None
    in_names, out_names, out_avals = [], [], []
    zero_out_shapes = []
    for alloc in nc.m.functions[0].allocations:
        if not isinstance(alloc, mybir.MemoryLocationSet):
            continue
        name = alloc.memorylocations[0].name
        if alloc.kind == "ExternalInput":
            if name != partition_name:
                in_names.append(name)
        elif alloc.kind == "ExternalOutput":
            shape = tuple(alloc.tensor_shape)
            dtype = mybir.dt.np(alloc.dtype)
            out_names.append(name)
            out_avals.append(jax.core.ShapedArray(shape, dtype))
            zero_out_shapes.append((shape, dtype))
    n_params = len(in_names)
    n_outs = len(out_names)
    all_names = list(in_names) + list(out_names)
    if partition_name is not None:
        all_names.append(partition_name)
    donate = tuple(range(n_params, n_params + n_outs))

    def _body(*args):
        operands = list(args)
        if partition_name is not None:
            operands.append(bass2jax.partition_id_tensor())
        outs = bass2jax._bass_exec_p.bind(
            *operands,
            out_avals=tuple(out_avals),
            in_names=tuple(all_names),
            out_names=tuple(out_names),
            lowering_input_output_aliases=(),
            sim_require_finite=True,
            sim_require_nnan=True,
            nc=nc,
        )
        return tuple(outs)

    devices = jax.devices()[:NCORES]
    assert len(devices) == NCORES, f"need {NCORES} devices, have {len(jax.devices())}"
    mesh = bass2jax.Mesh(np.asarray(devices), ("core",))
    in_specs = (bass2jax.PartitionSpec("core"),) * (n_params + n_outs)
    out_specs = (bass2jax.PartitionSpec("core"),) * n_outs
    sharded = jax.jit(
        bass2jax.shard_map(
            _body, mesh=mesh, in_specs=in_specs, out_specs=out_specs, check_rep=False
        ),
        donate_argnums=donate,
        keep_unused=True,
    )

    def run_fn(in_maps):
        concat_in = [
            np.concatenate([np.asarray(m[name]) for m in in_maps], axis=0)
            for name in in_names
        ]
        zeros = [
            np.zeros((shape[0] * NCORES,) + shape[1:], dtype)
            for shape, dtype in zero_out_shapes
        ]
        out_arrs = sharded(*concat_in, *zeros)
        results = []
        for c in range(NCORES):
            res = {}
            for i, name in enumerate(out_names):
                arr = np.asarray(out_arrs[i])
                per = arr.shape[0] // NCORES
                res[name] = arr[c * per:(c + 1) * per]
            results.append(res)
        return results

    return nc, run_fn


def _route(x, w_router):
    """Top-2 routing: expert ids + softmax weights, matching jax.lax.top_k
    (descending, ties to the lower index) + jax.nn.softmax."""
    logits = x.astype(np.float64) @ w_router.astype(np.float64)
    top2 = np.argsort(-logits, axis=1, kind="stable")[:, :TOPK]
    vals = np.take_along_axis(logits, top2, 1).astype(np.float32)
    e = np.exp(vals - vals.max(axis=1, keepdims=True))
    w = (e / e.sum(axis=1, keepdims=True)).astype(np.float32)
    return top2, w


def _reference_numpy(x, w_router, w_gate, w_up, w_down):
    """Correct-but-slow dense fallback for shapes the device program doesn't cover."""
    x = x.astype(np.float32)
    logits = x @ w_router.astype(np.float32)
    n_exp = w_gate.shape[0]
    k = min(TOPK, n_exp)
    top = np.argsort(-logits, axis=1, kind="stable")[:, :k]
    vals = np.take_along_axis(logits, top, 1)
    ex = np.exp(vals - vals.max(1, keepdims=True))
    ww = (ex / ex.sum(1, keepdims=True)).astype(np.float32)
    w_dense = np.zeros_like(logits)
    t_ids = np.arange(x.shape[0])[:, None]
    w_dense[t_ids, top] = ww
    out = np.zeros((x.shape[0], w_down.shape[-1]), np.float32)
    for e in range(n_exp):
        g = x @ w_gate[e]
        u = x @ w_up[e]
        a = (g / (1.0 + np.exp(-g))) * u
        out += w_dense[:, e:e + 1] * (a @ w_down[e])
    return out


def _pack_core_inputs(x, wg_e, wu_e, wd_e, toks, ws, nb1=0):
    """Build one core's input map: gathered/transposed tokens, packed
    gate|up weight tiles ([NMC, KT, 128, 512] matching the SBUF layout so
    each tile is one contiguous DMA), tiled w_down, routing weights.
    Everything fed to the PE is converted to bf16 on the host."""
    assert nb1 == 0
    C = B0
    NT = C // 128
    n_e = len(toks)
    xg = np.zeros((C, H), np.float32)
    xg[:n_e] = x[toks]
    xT = np.ascontiguousarray(xg.T).reshape(KT, 128, C).astype(BF16)
    rfull = np.zeros(C, np.float32)
    rfull[:n_e] = ws
    wgu = np.empty((NMC, KT, 128, 2 * CHUNK), BF16)
    wgu[..., :CHUNK] = wg_e.reshape(KT, 128, NMC, CHUNK).transpose(2, 0, 1, 3).astype(BF16)
    wgu[..., CHUNK:] = wu_e.reshape(KT, 128, NMC, CHUNK).transpose(2, 0, 1, 3).astype(BF16)
    wdt = np.ascontiguousarray(
        wd_e.reshape(KI, 128, NH, 512).transpose(2, 0, 1, 3)
    ).astype(BF16)
    return {
        "xT0": xT,
        "wgu": wgu,
        "wd": wdt,
        "rw": np.ascontiguousarray(rfull.reshape(NT, 128).T),
    }


def kernel(x, w_router, w_gate, w_up, w_down):
    x = np.ascontiguousarray(np.asarray(x, dtype=np.float32))
    w_router = np.asarray(w_router, dtype=np.float32)
    w_gate = np.ascontiguousarray(np.asarray(w_gate, dtype=np.float32))
    w_up = np.ascontiguousarray(np.asarray(w_up, dtype=np.float32))
    w_down = np.ascontiguousarray(np.asarray(w_down, dtype=np.float32))

    if (x.shape != (T, H) or w_router.shape != (H, E)
            or w_gate.shape != (E, H, II) or w_up.shape != (E, H, II)
            or w_down.shape != (E, II, H)):
        return _reference_numpy(x, w_router, w_gate, w_up, w_down)

    top2, w = _route(x, w_router)
    tok = np.repeat(np.arange(T), TOPK)
    te = top2.ravel()
    tw = w.ravel()
    toks_e, ws_e = [], []
    for e in range(E):
        sel = te == e
        toks_e.append(tok[sel])
        ws_e.append(tw[sel].astype(np.float32))

    # Capacity-factor dispatch: the device program handles up to B0=512
    # tokens per expert (98.5% of routed tokens for balanced routing); the
    # rare spill beyond capacity goes through an exact fp32 host path.
    nc, run_fn = _get_exec(0)

    in_maps = [
        _pack_core_inputs(x, w_gate[e], w_up[e], w_down[e],
                          toks_e[e][:B0], ws_e[e][:B0], 0)
        for e in range(E)
    ]

    try:
        results = run_fn(in_maps)
    except Exception:
        import time as _time
        _time.sleep(20)
        results = run_fn(in_maps)

    out = np.zeros((T, H), np.float32)
    for e in range(E):
        n_e = min(len(toks_e[e]), B0)
        d = results[e]["d"].reshape(B0, H)
        out[toks_e[e][:B0]] += d[:n_e]
        spill = toks_e[e][B0:]
        if spill.size:
            xe = x[spill]
            g = xe @ w_gate[e]
            u = xe @ w_up[e]
            a = (g / (1.0 + np.exp(-g))) * u
            out[spill] += (a @ w_down[e]) * ws_e[e][B0:, None]
    return out


# revision 6
# speedup vs baseline: 1.0173x; 1.0173x over previous
"""Trainium2 Bass kernel for nn_BlockSparseMLP (MoE gated MLP, E=8, top-2).

Strategy: expert parallelism over 8 NeuronCores. The router matmul
(x @ w_router, 67 MFLOP out of the 206 GFLOP total) plus the top-2
dispatch/gather and the final scatter-add combine run on the host; each
core runs the full gated MLP (gate/up, silu*up, down, weighted by the
routing prob) for the tokens routed to its expert.

All matmul operands are bf16 (host-converted; PSUM accumulation stays
fp32): full PE rate (1 row/cycle) like fp32r, half the HBM traffic.
Weights are streamed as contiguous 1 MB transfers ([128 part, 8 k-tiles,
512] halves, packed on the host so partition-major rows are 8 KB runs) —
128 KB tile-at-a-time DMA only sustains ~75 GB/s/queue and starved the
PE at startup; 1 MB transfers run at ~340 GB/s and keep the weight
stream ahead of the PE for the whole kernel.

Per-core device layout (capacity C = 512 tokens):
  phase 1 (gate/up): per I-chunk of 256, two 1 MB weight halves
    (gate|up packed side by side); weights stationary, xT moving
    (N=512). silu(gate)*up fused on ACT+DVE into aT ([I, C] bf16,
    SBUF-resident).
  phase 2 (down): stream w_down as 1 MB halves (moving [128,512]
    slices), aT tiles stationary, accumulate over I into [tokens, 512]
    psum tiles, scale by the routing weight on DVE, contiguous 256 KB
    fp32 stores on the HWDGE queues.
"""

import sys
import functools

sys.path.insert(0, "/opt/trn_rl_repo")

import numpy as np
import ml_dtypes

BF16 = ml_dtypes.bfloat16

T, H, II, E, TOPK = 2048, 2048, 4096, 8, 2
NCORES = 8
B0 = 512        # per-expert token capacity (moving N)
CHUNK = 256     # phase-1 I-chunk width
KT = H // 128   # 16 contraction tiles for gate/up
MTI = II // 128  # 32 I tiles
NMC = II // CHUNK  # 16 phase-1 chunks
JJ = CHUNK // 128  # 2 m-tiles per chunk
KI = II // 128  # 32 contraction tiles for down
NH = H // 512   # 4 output column chunks
NT = B0 // 128  # 4 token tiles
XQ = 4          # x is loaded as 4 quarter tiles of 4 k-slices each
WHK = 8         # k-tiles per 1 MB weight half


@functools.lru_cache(maxsize=2)
def _build(nb1: int = 0):
    """Build the SPMD Bass program (capacity B0 tokens; nb1 kept for
    test.py signature compat and must be 0 — overflow spills to host)."""
    assert nb1 == 0
    import concourse.mybir as mybir
    import concourse.tile as tile
    from concourse import bacc

    f32 = mybir.dt.float32
    bf16 = mybir.dt.bfloat16

    nc = bacc.Bacc(None)
    xT0 = nc.declare_dram_parameter("xT0", [XQ, 128, KT // XQ, B0], bf16, isOutput=False)
    wgu = nc.declare_dram_parameter("wgu", [NMC, 2, 128, WHK, 2 * CHUNK], bf16, isOutput=False)
    wd = nc.declare_dram_parameter("wd", [NH, KI // WHK, 128, WHK, 512], bf16, isOutput=False)
    rw = nc.declare_dram_parameter("rw", [128, NT], f32, isOutput=False)
    dout = nc.declare_dram_parameter("d", [NH, NT, 128, 512], f32, isOutput=True)

    SILU = mybir.ActivationFunctionType.Silu

    with tile.TileContext(nc) as tc:
        with (
            tc.tile_pool(name="pers", bufs=1) as pers,
            tc.tile_pool(name="wpool", bufs=12) as wpool,
        ):
            aT0 = pers.tile([128, MTI, B0], bf16)
            rwt = pers.tile([128, NT], f32)
            nc.gpsimd.dma_start(rwt[:], rw[:])

            with (
                tc.tile_pool(name="xp", bufs=1) as xp,
                tc.tile_pool(name="ps1", bufs=1, space="PSUM") as ps1,
                tc.tile_pool(name="sp", bufs=2) as sp,
            ):
                # x quarters stream on sync while the first weight halves
                # stream on scalar, so the PE's k-loop inputs arrive in
                # consumption order on independent queues.
                xq = []
                for q in range(XQ):
                    xt = xp.tile([128, KT // XQ, B0], bf16, name=f"xq{q}")
                    nc.sync.dma_start(xt[:], xT0[q])
                    xq.append(xt)

                for mc in range(NMC):
                    wb = []
                    for h in range(2):
                        hidx = mc * 2 + h
                        eng = nc.scalar if hidx % 2 == 0 else nc.sync
                        wbh = wpool.tile([128, WHK, 2 * CHUNK], bf16, name="wbh", tag="w")
                        eng.dma_start(wbh[:], wgu[mc, h])
                        wb.append(wbh)

                    pg0 = [ps1.tile([128, B0], f32, name="pg0", tag="pg0", bufs=3)
                           for _ in range(JJ)]
                    pu0 = [ps1.tile([128, B0], f32, name="pu0", tag="pu0", bufs=3)
                           for _ in range(JJ)]

                    for k in range(KT):
                        st = dict(start=(k == 0), stop=(k == KT - 1))
                        wk = wb[k // WHK][:, k % WHK, :]
                        xk = xq[k // (KT // XQ)][:, k % (KT // XQ), :]
                        for j in range(JJ):
                            nc.tensor.matmul(
                                pg0[j][:], wk[:, j * 128:(j + 1) * 128], xk, **st
                            )
                        for j in range(JJ):
                            nc.tensor.matmul(
                                pu0[j][:],
                                wk[:, CHUNK + j * 128:CHUNK + (j + 1) * 128],
                                xk, **st
                            )

                    for j in range(JJ):
                        m = mc * JJ + j
                        sg = sp.tile([128, B0], f32, name="sg", tag="sg")
                        nc.scalar.activation(sg[:], pg0[j][:], SILU)
                        nc.vector.tensor_mul(aT0[:, m, :], sg[:], pu0[j][:])

            with (
                tc.tile_pool(name="ps2", bufs=1, space="PSUM") as ps2,
                tc.tile_pool(name="op", bufs=6) as op,
            ):
                for nh in range(NH):
                    pd = [ps2.tile([128, 512], f32, name="pd", tag="pd", bufs=7)
                          for _ in range(NT)]
                    for h in range(KI // WHK):
                        hidx = nh * (KI // WHK) + h
                        eng = nc.scalar if hidx % 2 == 0 else nc.sync
                        wdh = wpool.tile([128, WHK, 512], bf16, name="wdh", tag="w")
                        eng.dma_start(wdh[:], wd[nh, h])
                        for kk in range(WHK):
                            k = h * WHK + kk
                            st = dict(start=(k == 0), stop=(k == KI - 1))
                            for mt in range(NT):
                                nc.tensor.matmul(
                                    pd[mt][:], aT0[:, k, mt * 128:(mt + 1) * 128],
                                    wdh[:, kk, :], **st
                                )
                    for mt in range(NT):
                        ot = op.tile([128, 512], f32, name="ot", tag="ot")
                        nc.vector.tensor_scalar_mul(ot[:], pd[mt][:], rwt[:, mt:mt + 1])
                        eng = nc.sync if (nh * NT + mt) % 2 == 0 else nc.scalar
                        eng.dma_start(dout[nh, mt], ot[:])

    nc.compile()
    return nc


@functools.lru_cache(maxsize=2)
def _get_exec(nb1: int = 0):
    """Compile the Bass program and return (nc, run_fn) with a cached jit.

    run_fn(in_maps) -> list of per-core {"d": np.ndarray}. Mirrors
    bass2jax.run_bass_via_pjrt's multi-core branch, but keeps the jitted
    function alive across kernel() calls so repeat invocations skip XLA
    and NEFF compilation.
    """
    import jax
    import concourse.mybir as mybir
    from concourse import bass2jax

    nc = _build(nb1)
    bass2jax.install_neuronx_cc_hook()

    partition_name = nc.partition_id_tensor.name if nc.partition_id_tensor else None
    in_names, out_names, out_avals = [], [], []
    zero_out_shapes = []
    for alloc in nc.m.functions[0].allocations:
        if not isinstance(alloc, mybir.MemoryLocationSet):
            continue
        name = alloc.memorylocations[0].name
        if alloc.kind == "ExternalInput":
            if name != partition_name:
                in_names.append(name)
        elif alloc.kind == "ExternalOutput":
            shape = tuple(alloc.tensor_shape)
            dtype = mybir.dt.np(alloc.dtype)
            out_names.append(name)
            out_avals.append(jax.core.ShapedArray(shape, dtype))
            zero_out_shapes.append((shape, dtype))
    n_params = len(in_names)
    n_outs = len(out_names)
    all_names = list(in_names) + list(out_names)
    if partition_name is not None:
        all_names.append(partition_name)
    donate = tuple(range(n_params, n_params + n_outs))

    def _body(*args):
        operands = list(args)
        if partition_name is not None:
            operands.append(bass2jax.partition_id_tensor())
        outs = bass2jax._bass_exec_p.bind(
            *operands,
            out_avals=tuple(out_avals),
            in_names=tuple(all_names),
            out_names=tuple(out_names),
            lowering_input_output_aliases=(),
            sim_require_finite=True,
            sim_require_nnan=True,
            nc=nc,
        )
        return tuple(outs)

    devices = jax.devices()[:NCORES]
    assert len(devices) == NCORES, f"need {NCORES} devices, have {len(jax.devices())}"
    mesh = bass2jax.Mesh(np.asarray(devices), ("core",))
    in_specs = (bass2jax.PartitionSpec("core"),) * (n_params + n_outs)
    out_specs = (bass2jax.PartitionSpec("core"),) * n_outs
    sharded = jax.jit(
        bass2jax.shard_map(
            _body, mesh=mesh, in_specs=in_specs, out_specs=out_specs, check_rep=False
        ),
        donate_argnums=donate,
        keep_unused=True,
    )

    def run_fn(in_maps):
        concat_in = [
            np.concatenate([np.asarray(m[name]) for m in in_maps], axis=0)
            for name in in_names
        ]
        zeros = [
            np.zeros((shape[0] * NCORES,) + shape[1:], dtype)
            for shape, dtype in zero_out_shapes
        ]
        out_arrs = sharded(*concat_in, *zeros)
        results = []
        for c in range(NCORES):
            res = {}
            for i, name in enumerate(out_names):
                arr = np.asarray(out_arrs[i])
                per = arr.shape[0] // NCORES
                res[name] = arr[c * per:(c + 1) * per]
            results.append(res)
        return results

    return nc, run_fn


def _route(x, w_router):
    """Top-2 routing: expert ids + softmax weights, matching jax.lax.top_k
    (descending, ties to the lower index) + jax.nn.softmax."""
    logits = x.astype(np.float64) @ w_router.astype(np.float64)
    top2 = np.argsort(-logits, axis=1, kind="stable")[:, :TOPK]
    vals = np.take_along_axis(logits, top2, 1).astype(np.float32)
    e = np.exp(vals - vals.max(axis=1, keepdims=True))
    w = (e / e.sum(axis=1, keepdims=True)).astype(np.float32)
    return top2, w


def _reference_numpy(x, w_router, w_gate, w_up, w_down):
    """Correct-but-slow dense fallback for shapes the device program doesn't cover."""
    x = x.astype(np.float32)
    logits = x @ w_router.astype(np.float32)
    n_exp = w_gate.shape[0]
    k = min(TOPK, n_exp)
    top = np.argsort(-logits, axis=1, kind="stable")[:, :k]
    vals = np.take_along_axis(logits, top, 1)
    ex = np.exp(vals - vals.max(1, keepdims=True))
    ww = (ex / ex.sum(1, keepdims=True)).astype(np.float32)
    w_dense = np.zeros_like(logits)
    t_ids = np.arange(x.shape[0])[:, None]
    w_dense[t_ids, top] = ww
    out = np.zeros((x.shape[0], w_down.shape[-1]), np.float32)
    for e in range(n_exp):
        g = x @ w_gate[e]
        u = x @ w_up[e]
        a = (g / (1.0 + np.exp(-g))) * u
        out += w_dense[:, e:e + 1] * (a @ w_down[e])
    return out


def _pack_core_inputs(x, wg_e, wu_e, wd_e, toks, ws, nb1=0):
    """Build one core's input map. Everything fed to the PE is converted
    to bf16 and laid out partition-major so every weight half is one
    contiguous 1 MB DMA ([128, 8 k-tiles, 512] with 8 KB rows) and x is
    four contiguous 512 KB quarters."""
    assert nb1 == 0
    n_e = len(toks)
    xg = np.zeros((B0, H), np.float32)
    xg[:n_e] = x[toks]
    # xT[k, p, t] = xg[t, k*128+p] -> quarters [q, p, kk, t]
    xT = np.ascontiguousarray(xg.T).reshape(KT, 128, B0).astype(BF16)
    xTq = np.ascontiguousarray(
        xT.reshape(XQ, KT // XQ, 128, B0).transpose(0, 2, 1, 3)
    )
    rfull = np.zeros(B0, np.float32)
    rfull[:n_e] = ws
    # wgu[mc, half, p, kk, 0:256 | 256:512] = gate|up[(half*8+kk)*128+p, mc-chunk]
    wgu = np.empty((NMC, 2, 128, WHK, 2 * CHUNK), BF16)
    wgu[..., :CHUNK] = (
        wg_e.reshape(2, WHK, 128, NMC, CHUNK).transpose(3, 0, 2, 1, 4).astype(BF16)
    )
    wgu[..., CHUNK:] = (
        wu_e.reshape(2, WHK, 128, NMC, CHUNK).transpose(3, 0, 2, 1, 4).astype(BF16)
    )
    # wd[nh, half, p, kk, h] = w_down[(half*8+kk)*128+p, nh*512+h]
    wdt = np.ascontiguousarray(
        wd_e.reshape(KI // WHK, WHK, 128, NH, 512).transpose(3, 0, 2, 1, 4)
    ).astype(BF16)
    return {
        "xT0": xTq,
        "wgu": wgu,
        "wd": wdt,
        "rw": np.ascontiguousarray(rfull.reshape(NT, 128).T),
    }


def kernel(x, w_router, w_gate, w_up, w_down):
    x = np.ascontiguousarray(np.asarray(x, dtype=np.float32))
    w_router = np.asarray(w_router, dtype=np.float32)
    w_gate = np.ascontiguousarray(np.asarray(w_gate, dtype=np.float32))
    w_up = np.ascontiguousarray(np.asarray(w_up, dtype=np.float32))
    w_down = np.ascontiguousarray(np.asarray(w_down, dtype=np.float32))

    if (x.shape != (T, H) or w_router.shape != (H, E)
            or w_gate.shape != (E, H, II) or w_up.shape != (E, H, II)
            or w_down.shape != (E, II, H)):
        return _reference_numpy(x, w_router, w_gate, w_up, w_down)

    top2, w = _route(x, w_router)
    tok = np.repeat(np.arange(T), TOPK)
    te = top2.ravel()
    tw = w.ravel()
    toks_e, ws_e = [], []
    for e in range(E):
        sel = te == e
        toks_e.append(tok[sel])
        ws_e.append(tw[sel].astype(np.float32))

    # Capacity-factor dispatch: the device program handles up to B0=512
    # tokens per expert (98.5% of routed tokens for balanced routing); the
    # rare spill beyond capacity goes through an exact fp32 host path.
    nc, run_fn = _get_exec(0)

    in_maps = [
        _pack_core_inputs(x, w_gate[e], w_up[e], w_down[e],
                          toks_e[e][:B0], ws_e[e][:B0], 0)
        for e in range(E)
    ]

    try:
        results = run_fn(in_maps)
    except Exception:
        import time as _time
        _time.sleep(20)
        results = run_fn(in_maps)

    out = np.zeros((T, H), np.float32)
    for e in range(E):
        n_e = min(len(toks_e[e]), B0)
        # d: [NH, NT, 128, 512] -> [NT*128 tokens, NH*512 hidden]
        d = results[e]["d"].transpose(1, 2, 0, 3).reshape(B0, H)
        out[toks_e[e][:B0]] += d[:n_e]
        spill = toks_e[e][B0:]
        if spill.size:
            xe = x[spill]
            g = xe @ w_gate[e]
            u = xe @ w_up[e]
            a = (g / (1.0 + np.exp(-g))) * u
            out[spill] += (a @ w_down[e]) * ws_e[e][B0:, None]
    return out


# revision 9
# speedup vs baseline: 1.0415x; 1.0237x over previous
"""Trainium2 Bass kernel for nn_BlockSparseMLP (MoE gated MLP, E=8, top-2).

Strategy: expert parallelism over 8 NeuronCores. The router matmul
(x @ w_router, 67 MFLOP out of the 206 GFLOP total) plus the top-2
dispatch/gather and the final scatter-add combine run on the host; each
core runs the full gated MLP (gate/up, silu*up, down, weighted by the
routing prob) for the tokens routed to its expert.

All matmul operands are bf16 (host-converted; PSUM accumulation stays
fp32): full PE rate (1 row/cycle) like fp32r, half the HBM traffic.
Weights are streamed as contiguous 1 MB transfers ([128 part, 8 k-tiles,
512] halves, packed on the host so partition-major rows are 8 KB runs) —
128 KB tile-at-a-time DMA only sustains ~75 GB/s/queue and starved the
PE at startup; 1 MB transfers run at ~340 GB/s and keep the weight
stream ahead of the PE for the whole kernel.

Per-core device layout (capacity C = 512 tokens):
  phase 1 (gate/up): per I-chunk of 256, two 1 MB weight halves
    (gate|up packed side by side); weights stationary, xT moving
    (N=512). silu(gate)*up fused on ACT+DVE into aT ([I, C] bf16,
    SBUF-resident).
  phase 2 (down): stream w_down as 1 MB halves (moving [128,512]
    slices), aT tiles stationary, accumulate over I into [tokens, 512]
    psum tiles, scale by the routing weight on DVE, contiguous 256 KB
    fp32 stores on the HWDGE queues.
"""

import sys
import functools

sys.path.insert(0, "/opt/trn_rl_repo")

import numpy as np
import ml_dtypes

BF16 = ml_dtypes.bfloat16

T, H, II, E, TOPK = 2048, 2048, 4096, 8, 2
NCORES = 8
B0 = 512        # per-expert token capacity (moving N)
CHUNK = 256     # phase-1 I-chunk width
KT = H // 128   # 16 contraction tiles for gate/up
MTI = II // 128  # 32 I tiles
NMC = II // CHUNK  # 16 phase-1 chunks
JJ = CHUNK // 128  # 2 m-tiles per chunk
KI = II // 128  # 32 contraction tiles for down
NH = H // 512   # 4 output column chunks
NT = B0 // 128  # 4 token tiles
XQ = 4          # x is loaded as 4 quarter tiles of 4 k-slices each
WHK = 8         # k-tiles per 1 MB weight half


@functools.lru_cache(maxsize=2)
def _build(nb1: int = 0):
    """Build the SPMD Bass program (capacity B0 tokens; nb1 kept for
    test.py signature compat and must be 0 — overflow spills to host)."""
    assert nb1 == 0
    import concourse.mybir as mybir
    import concourse.tile as tile
    from concourse import bacc

    f32 = mybir.dt.float32
    bf16 = mybir.dt.bfloat16

    nc = bacc.Bacc(None)
    xT0 = nc.declare_dram_parameter("xT0", [XQ, 128, KT // XQ, B0], bf16, isOutput=False)
    wgu = nc.declare_dram_parameter("wgu", [NMC, 2, 128, WHK, 2 * CHUNK], bf16, isOutput=False)
    wd = nc.declare_dram_parameter("wd", [NH, KI // WHK, 128, WHK, 512], bf16, isOutput=False)
    rw = nc.declare_dram_parameter("rw", [128, NT], f32, isOutput=False)
    dout = nc.declare_dram_parameter("d", [NH, NT, 128, 512], f32, isOutput=True)

    SILU = mybir.ActivationFunctionType.Silu

    with tile.TileContext(nc) as tc:
        with (
            tc.tile_pool(name="pers", bufs=1) as pers,
            tc.tile_pool(name="wpool", bufs=10) as wpool,
        ):
            aT0 = pers.tile([128, MTI, B0], bf16)
            rwt = pers.tile([128, NT], f32)

            with (
                tc.tile_pool(name="xp", bufs=1) as xp,
                tc.tile_pool(name="ps1", bufs=1, space="PSUM") as ps1,
                tc.tile_pool(name="sp", bufs=2) as sp,
            ):
                # Startup choreography: xq0 leads the sync HWDGE ring so the
                # first matmul's moving operand lands first; the remaining x
                # quarters ride the gpsimd (SWDGE) ring so no weight half
                # queues behind 2 MB of x. Chunk 0's weights arrive as
                # 256 KB quarter-tiles (first matmul ~2 us earlier than
                # waiting on a full 1 MB half); later chunks stream as 1 MB
                # halves alternating scalar/sync.
                xq = []
                for q in range(XQ):
                    xt = xp.tile([128, KT // XQ, B0], bf16, name=f"xq{q}")
                    (nc.sync if q == 0 else nc.gpsimd).dma_start(xt[:], xT0[q])
                    xq.append(xt)
                nc.gpsimd.dma_start(rwt[:], rw[:])

                for mc in range(NMC):
                    if mc == 0:
                        wq = []
                        for i in range(8):
                            wt = xp.tile([128, 2, 2 * CHUNK], bf16, name=f"wq{i}")
                            eng = nc.scalar if i % 2 == 0 else nc.sync
                            eng.dma_start(
                                wt[:],
                                wgu[0, i // 4][:, (i % 4) * 2:(i % 4) * 2 + 2, :],
                            )
                            wq.append(wt)
                        wk_of = lambda k: wq[k // 2][:, k % 2, :]
                    else:
                        wb = []
                        for h in range(2):
                            hidx = mc * 2 + h
                            eng = nc.scalar if hidx % 2 == 0 else nc.sync
                            wbh = wpool.tile([128, WHK, 2 * CHUNK], bf16, name="wbh", tag="w")
                            eng.dma_start(wbh[:], wgu[mc, h])
                            wb.append(wbh)
                        wk_of = lambda k, wb=wb: wb[k // WHK][:, k % WHK, :]

                    pg0 = [ps1.tile([128, B0], f32, name="pg0", tag="pg0", bufs=3)
                           for _ in range(JJ)]
                    pu0 = [ps1.tile([128, B0], f32, name="pu0", tag="pu0", bufs=3)
                           for _ in range(JJ)]

                    for k in range(KT):
                        st = dict(start=(k == 0), stop=(k == KT - 1))
                        wk = wk_of(k)
                        xk = xq[k // (KT // XQ)][:, k % (KT // XQ), :]
                        for j in range(JJ):
                            nc.tensor.matmul(
                                pg0[j][:], wk[:, j * 128:(j + 1) * 128], xk, **st
                            )
                        for j in range(JJ):
                            nc.tensor.matmul(
                                pu0[j][:],
                                wk[:, CHUNK + j * 128:CHUNK + (j + 1) * 128],
                                xk, **st
                            )

                    for j in range(JJ):
                        m = mc * JJ + j
                        sg = sp.tile([128, B0], f32, name="sg", tag="sg")
                        nc.scalar.activation(sg[:], pg0[j][:], SILU)
                        nc.vector.tensor_mul(aT0[:, m, :], sg[:], pu0[j][:])

            with (
                tc.tile_pool(name="ps2", bufs=1, space="PSUM") as ps2,
                tc.tile_pool(name="op", bufs=6) as op,
            ):
                for nh in range(NH):
                    pd = [ps2.tile([128, 512], f32, name="pd", tag="pd", bufs=7)
                          for _ in range(NT)]
                    for h in range(KI // WHK):
                        hidx = nh * (KI // WHK) + h
                        eng = nc.scalar if hidx % 2 == 0 else nc.sync
                        wdh = wpool.tile([128, WHK, 512], bf16, name="wdh", tag="w")
                        eng.dma_start(wdh[:], wd[nh, h])
                        for kk in range(WHK):
                            k = h * WHK + kk
                            st = dict(start=(k == 0), stop=(k == KI - 1))
                            for mt in range(NT):
                                nc.tensor.matmul(
                                    pd[mt][:], aT0[:, k, mt * 128:(mt + 1) * 128],
                                    wdh[:, kk, :], **st
                                )
                    for mt in range(NT):
                        ot = op.tile([128, 512], f32, name="ot", tag="ot")
                        # alternate DVE/ACT so the last group's four scales
                        # drain on two engines instead of serializing on DVE
                        if mt % 2 == 0:
                            nc.vector.tensor_scalar_mul(ot[:], pd[mt][:], rwt[:, mt:mt + 1])
                        else:
                            nc.scalar.mul(ot[:], pd[mt][:], rwt[:, mt:mt + 1])
                        eng = nc.sync if (nh * NT + mt) % 2 == 0 else nc.scalar
                        eng.dma_start(dout[nh, mt], ot[:])

    nc.compile()
    return nc


@functools.lru_cache(maxsize=2)
def _get_exec(nb1: int = 0):
    """Compile the Bass program and return (nc, run_fn) with a cached jit.

    run_fn(in_maps) -> list of per-core {"d": np.ndarray}. Mirrors
    bass2jax.run_bass_via_pjrt's multi-core branch, but keeps the jitted
    function alive across kernel() calls so repeat invocations skip XLA
    and NEFF compilation.
    """
    import jax
    import concourse.mybir as mybir
    from concourse import bass2jax

    nc = _build(nb1)
    bass2jax.install_neuronx_cc_hook()

    partition_name = nc.partition_id_tensor.name if nc.partition_id_tensor else None
    in_names, out_names, out_avals = [], [], []
    zero_out_shapes = []
    for alloc in nc.m.functions[0].allocations:
        if not isinstance(alloc, mybir.MemoryLocationSet):
            continue
        name = alloc.memorylocations[0].name
        if alloc.kind == "ExternalInput":
            if name != partition_name:
                in_names.append(name)
        elif alloc.kind == "ExternalOutput":
            shape = tuple(alloc.tensor_shape)
            dtype = mybir.dt.np(alloc.dtype)
            out_names.append(name)
            out_avals.append(jax.core.ShapedArray(shape, dtype))
            zero_out_shapes.append((shape, dtype))
    n_params = len(in_names)
    n_outs = len(out_names)
    all_names = list(in_names) + list(out_names)
    if partition_name is not None:
        all_names.append(partition_name)
    donate = tuple(range(n_params, n_params + n_outs))

    def _body(*args):
        operands = list(args)
        if partition_name is not None:
            operands.append(bass2jax.partition_id_tensor())
        outs = bass2jax._bass_exec_p.bind(
            *operands,
            out_avals=tuple(out_avals),
            in_names=tuple(all_names),
            out_names=tuple(out_names),
            lowering_input_output_aliases=(),
            sim_require_finite=True,
            sim_require_nnan=True,
            nc=nc,
        )
        return tuple(outs)

    devices = jax.devices()[:NCORES]
    assert len(devices) == NCORES, f"need {NCORES} devices, have {len(jax.devices())}"
    mesh = bass2jax.Mesh(np.asarray(devices), ("core",))
    in_specs = (bass2jax.PartitionSpec("core"),) * (n_params + n_outs)
    out_specs = (bass2jax.PartitionSpec("core"),) * n_outs
    sharded = jax.jit(
        bass2jax.shard_map(
            _body, mesh=mesh, in_specs=in_specs, out_specs=out_specs, check_rep=False
        ),
        donate_argnums=donate,
        keep_unused=True,
    )

    def run_fn(in_maps):
        concat_in = [
            np.concatenate([np.asarray(m[name]) for m in in_maps], axis=0)
            for name in in_names
        ]
        zeros = [
            np.zeros((shape[0] * NCORES,) + shape[1:], dtype)
            for shape, dtype in zero_out_shapes
        ]
        out_arrs = sharded(*concat_in, *zeros)
        results = []
        for c in range(NCORES):
            res = {}
            for i, name in enumerate(out_names):
                arr = np.asarray(out_arrs[i])
                per = arr.shape[0] // NCORES
                res[name] = arr[c * per:(c + 1) * per]
            results.append(res)
        return results

    return nc, run_fn


def _route(x, w_router):
    """Top-2 routing: expert ids + softmax weights, matching jax.lax.top_k
    (descending, ties to the lower index) + jax.nn.softmax."""
    logits = x.astype(np.float64) @ w_router.astype(np.float64)
    top2 = np.argsort(-logits, axis=1, kind="stable")[:, :TOPK]
    vals = np.take_along_axis(logits, top2, 1).astype(np.float32)
    e = np.exp(vals - vals.max(axis=1, keepdims=True))
    w = (e / e.sum(axis=1, keepdims=True)).astype(np.float32)
    return top2, w


def _reference_numpy(x, w_router, w_gate, w_up, w_down):
    """Correct-but-slow dense fallback for shapes the device program doesn't cover."""
    x = x.astype(np.float32)
    logits = x @ w_router.astype(np.float32)
    n_exp = w_gate.shape[0]
    k = min(TOPK, n_exp)
    top = np.argsort(-logits, axis=1, kind="stable")[:, :k]
    vals = np.take_along_axis(logits, top, 1)
    ex = np.exp(vals - vals.max(1, keepdims=True))
    ww = (ex / ex.sum(1, keepdims=True)).astype(np.float32)
    w_dense = np.zeros_like(logits)
    t_ids = np.arange(x.shape[0])[:, None]
    w_dense[t_ids, top] = ww
    out = np.zeros((x.shape[0], w_down.shape[-1]), np.float32)
    for e in range(n_exp):
        g = x @ w_gate[e]
        u = x @ w_up[e]
        a = (g / (1.0 + np.exp(-g))) * u
        out += w_dense[:, e:e + 1] * (a @ w_down[e])
    return out


def _pack_core_inputs(x, wg_e, wu_e, wd_e, toks, ws, nb1=0):
    """Build one core's input map. Everything fed to the PE is converted
    to bf16 and laid out partition-major so every weight half is one
    contiguous 1 MB DMA ([128, 8 k-tiles, 512] with 8 KB rows) and x is
    four contiguous 512 KB quarters."""
    assert nb1 == 0
    n_e = len(toks)
    xg = np.zeros((B0, H), np.float32)
    xg[:n_e] = x[toks]
    # xT[k, p, t] = xg[t, k*128+p] -> quarters [q, p, kk, t]
    xT = np.ascontiguousarray(xg.T).reshape(KT, 128, B0).astype(BF16)
    xTq = np.ascontiguousarray(
        xT.reshape(XQ, KT // XQ, 128, B0).transpose(0, 2, 1, 3)
    )
    rfull = np.zeros(B0, np.float32)
    rfull[:n_e] = ws
    # wgu[mc, half, p, kk, 0:256 | 256:512] = gate|up[(half*8+kk)*128+p, mc-chunk]
    wgu = np.empty((NMC, 2, 128, WHK, 2 * CHUNK), BF16)
    wgu[..., :CHUNK] = (
        wg_e.reshape(2, WHK, 128, NMC, CHUNK).transpose(3, 0, 2, 1, 4).astype(BF16)
    )
    wgu[..., CHUNK:] = (
        wu_e.reshape(2, WHK, 128, NMC, CHUNK).transpose(3, 0, 2, 1, 4).astype(BF16)
    )
    # wd[nh, half, p, kk, h] = w_down[(half*8+kk)*128+p, nh*512+h]
    wdt = np.ascontiguousarray(
        wd_e.reshape(KI // WHK, WHK, 128, NH, 512).transpose(3, 0, 2, 1, 4)
    ).astype(BF16)
    return {
        "xT0": xTq,
        "wgu": wgu,
        "wd": wdt,
        "rw": np.ascontiguousarray(rfull.reshape(NT, 128).T),
    }


def kernel(x, w_router, w_gate, w_up, w_down):
    x = np.ascontiguousarray(np.asarray(x, dtype=np.float32))
    w_router = np.asarray(w_router, dtype=np.float32)
    w_gate = np.ascontiguousarray(np.asarray(w_gate, dtype=np.float32))
    w_up = np.ascontiguousarray(np.asarray(w_up, dtype=np.float32))
    w_down = np.ascontiguousarray(np.asarray(w_down, dtype=np.float32))

    if (x.shape != (T, H) or w_router.shape != (H, E)
            or w_gate.shape != (E, H, II) or w_up.shape != (E, H, II)
            or w_down.shape != (E, II, H)):
        return _reference_numpy(x, w_router, w_gate, w_up, w_down)

    top2, w = _route(x, w_router)
    tok = np.repeat(np.arange(T), TOPK)
    te = top2.ravel()
    tw = w.ravel()
    toks_e, ws_e = [], []
    for e in range(E):
        sel = te == e
        toks_e.append(tok[sel])
        ws_e.append(tw[sel].astype(np.float32))

    # Capacity-factor dispatch: the device program handles up to B0=512
    # tokens per expert (98.5% of routed tokens for balanced routing); the
    # rare spill beyond capacity goes through an exact fp32 host path.
    nc, run_fn = _get_exec(0)

    in_maps = [
        _pack_core_inputs(x, w_gate[e], w_up[e], w_down[e],
                          toks_e[e][:B0], ws_e[e][:B0], 0)
        for e in range(E)
    ]

    try:
        results = run_fn(in_maps)
    except Exception:
        import time as _time
        _time.sleep(20)
        results = run_fn(in_maps)

    out = np.zeros((T, H), np.float32)
    for e in range(E):
        n_e = min(len(toks_e[e]), B0)
        # d: [NH, NT, 128, 512] -> [NT*128 tokens, NH*512 hidden]
        d = results[e]["d"].transpose(1, 2, 0, 3).reshape(B0, H)
        out[toks_e[e][:B0]] += d[:n_e]
        spill = toks_e[e][B0:]
        if spill.size:
            xe = x[spill]
            g = xe @ w_gate[e]
            u = xe @ w_up[e]
            a = (g / (1.0 + np.exp(-g))) * u
            out[spill] += (a @ w_down[e]) * ws_e[e][B0:, None]
    return out
